# revision 1
# baseline (speedup 1.0000x reference)
"""Trainium2 Bass kernel for GatedCrossAttention (B=4, N=4096, C=1024, H=16, M=4).

Reference math (dead code removed: the v/gate projections are overwritten
by views of k in the original module, so v = g = k):
    q = query @ Wq.T + bq                    [B,N,C]   -> [B,N,H,hd]
    k = key   @ Wk.T + bk                    [B,N,M,C] -> [B,N,M,H,hd]
    attn = softmax_M(SCALE * einsum('bnhc,bnmhc->bnmh', q, k))
    out  = einsum('bnmh,bnmhc->bnhc', attn, k*k) . reshape(B,N,C)
    out  = out @ Wo.T + bo

Strategy: pure data parallel over the 16384 tokens (8 cores x 2048), no
collectives.  On-chip layout is "transposed": channels on partitions, tokens
on the free axis, so every matmul contraction (over channels) is a natural
PE op.  The per-head segment reductions use indicator matmuls with the
elementwise product q*k as the stationary operand, landing logits token-major
([t, (m,h)]) so the M-softmax runs on all 128 partitions; a tiny PE transpose
of the softmax weights returns them head-major for the head->channel
broadcast matmuls.  Host pre-transposes/casts inputs and weights to fp16
(error vs f32 reference ~1e-3, PE runs 16-bit at full rate), accumulation
stays f32 in PSUM.
"""

import dataclasses
import numpy as np
from contextlib import ExitStack

try:
    import concourse.bass as bass
except ImportError:  # path fallback for bare containers
    import sys

    sys.path.insert(0, "/opt/trn_rl_repo")
    import concourse.bass as bass

import concourse.tile as tile
from concourse import bacc, mybir
from concourse.bass_utils import run_bass_kernel_spmd
from concourse.masks import make_identity

# problem constants (hardcoded per the task contract)
B, N, C, H, HD, M = 4, 4096, 1024, 16, 64, 4
SCALE = float(HD) ** -0.5
NCORES = 8
T_TOTAL = B * N
T_CORE = T_TOTAL // NCORES  # 2048
TB = 512                    # tokens per block (one PSUM bank at f32)
NJ = C // 128               # 8 channel tiles
NT = TB // 128              # 4 token subtiles per block

DT = mybir.dt.float16
NPDT = np.float16
F32 = mybir.dt.float32


def _bcast(ap, reps, axis):
    """Insert a 0-stride dim of size `reps` at AP position `axis` (0=partition)."""
    new = list(ap.ap)
    new.insert(axis, [0, reps])
    return dataclasses.replace(ap, ap=new)


def build_nc(t_core=T_CORE, with_bias=False):
    nblk = t_core // TB
    nc = bacc.Bacc("TRN2", target_bir_lowering=False, debug=False)

    qT = nc.declare_dram_parameter("qT", [C, t_core], DT, isOutput=False)
    kT = nc.declare_dram_parameter("kT", [M, C, t_core], DT, isOutput=False)
    wqT = nc.declare_dram_parameter("wqT", [C, C], DT, isOutput=False)
    wkT = nc.declare_dram_parameter("wkT", [C, C], DT, isOutput=False)
    woT = nc.declare_dram_parameter("woT", [C, C], DT, isOutput=False)
    indl = nc.declare_dram_parameter("indl", [128, 2], DT, isOutput=False)
    indb = nc.declare_dram_parameter("indb", [M, NJ, 64, 128], DT, isOutput=False)
    if with_bias:
        bq = nc.declare_dram_parameter("bq", [1, C], DT, isOutput=False)
        bk = nc.declare_dram_parameter("bk", [1, C], DT, isOutput=False)
        bo = nc.declare_dram_parameter("bo", [1, C], DT, isOutput=False)
    out = nc.declare_dram_parameter("out", [t_core, C], F32, isOutput=True)

    # DRAM views: channel dim split into (chunk, partition)
    qT_v = qT.ap().rearrange("(c p) t -> p c t", p=128)
    kT_v = kT.ap().rearrange("m (c p) t -> p m c t", p=128)
    wq_v = wqT.ap().rearrange("(c p) j -> p c j", p=128)
    wk_v = wkT.ap().rearrange("(c p) j -> p c j", p=128)
    wo_v = woT.ap().rearrange("(c p) j -> p c j", p=128)

    with tile.TileContext(nc) as tc, ExitStack() as ctx:
        consts = ctx.enter_context(tc.tile_pool(name="consts", bufs=1))
        p_inq = ctx.enter_context(tc.tile_pool(name="inq", bufs=2))
        p_ink = ctx.enter_context(tc.tile_pool(name="ink", bufs=2))
        p_qp = ctx.enter_context(tc.tile_pool(name="qp", bufs=9))
        p_kp = ctx.enter_context(tc.tile_pool(name="kp", bufs=8))
        p_prod = ctx.enter_context(tc.tile_pool(name="prod", bufs=2))
        p_ksq = ctx.enter_context(tc.tile_pool(name="ksq", bufs=2))
        p_sm = ctx.enter_context(tc.tile_pool(name="sm", bufs=2))
        p_ct = ctx.enter_context(tc.tile_pool(name="ct", bufs=2))
        p_y = ctx.enter_context(tc.tile_pool(name="y", bufs=2))
        p_yb = ctx.enter_context(tc.tile_pool(name="yb", bufs=10))
        p_out = ctx.enter_context(tc.tile_pool(name="outs", bufs=3))
        pp = ctx.enter_context(tc.tile_pool(name="pp", bufs=2, space="PSUM"))
        pl = ctx.enter_context(tc.tile_pool(name="pl", bufs=2, space="PSUM"))
        pw = ctx.enter_context(tc.tile_pool(name="pw", bufs=2, space="PSUM"))
        pb = ctx.enter_context(tc.tile_pool(name="pb", bufs=2, space="PSUM"))

        # ---- constants / weights (resident) ----
        wq_sb = consts.tile([128, NJ, C], DT)
        wk_sb = consts.tile([128, NJ, C], DT)
        wo_sb = consts.tile([128, NJ, C], DT)
        nc.sync.dma_start(out=wq_sb, in_=wq_v)
        nc.sync.dma_start(out=wk_sb, in_=wk_v)
        nc.sync.dma_start(out=wo_sb, in_=wo_v)
        indl_sb = consts.tile([128, 2], DT)
        indb_sb = consts.tile([64, M, NJ, 128], DT)
        nc.sync.dma_start(out=indl_sb, in_=indl.ap())
        nc.sync.dma_start(out=indb_sb, in_=indb.ap().rearrange("m r p j -> p m r j"))
        ident = consts.tile([128, 128], DT)
        make_identity(nc, ident)
        if with_bias:
            ones_sb = consts.tile([1, TB], DT)
            nc.vector.memset(ones_sb, 1.0)
            bq_sb = consts.tile([1, C], DT)
            bk_sb = consts.tile([1, C], DT)
            bo_sb = consts.tile([1, C], DT)
            nc.sync.dma_start(out=bq_sb, in_=bq.ap())
            nc.sync.dma_start(out=bk_sb, in_=bk.ap())
            nc.sync.dma_start(out=bo_sb, in_=bo.ap())

        for blk in range(nblk):
            t0 = blk * TB
            tsl = slice(t0, t0 + TB)

            # ---- load inputs ----
            q_in = p_inq.tile([128, NJ, TB], DT)
            nc.sync.dma_start(out=q_in, in_=qT_v[:, :, tsl])
            k_in = [p_ink.tile([128, NJ, TB], DT, tag="kin", name="kin")
                    for _ in range(M)]
            for m in range(M):
                nc.sync.dma_start(out=k_in[m], in_=kT_v[:, m, :, tsl])

            # ---- projections (PE) ----
            qp = [p_qp.tile([128, TB], DT, tag="qp", name="qp") for _ in range(NJ)]
            for r in range(NJ):
                ps = pp.tile([128, TB], F32, tag="pp", name="pp")
                for c in range(NJ):
                    nc.tensor.matmul(
                        ps,
                        wq_sb[:, c, r * 128:(r + 1) * 128],
                        q_in[:, c, :],
                        start=(c == 0),
                        stop=(c == NJ - 1 and not with_bias),
                    )
                if with_bias:
                    nc.tensor.matmul(
                        ps, bq_sb[:, r * 128:(r + 1) * 128], ones_sb,
                        start=False, stop=True,
                    )
                nc.scalar.copy(out=qp[r], in_=ps)

            kp = [p_kp.tile([128, M, TB], DT, tag="kp", name="kp")
                  for _ in range(NJ)]
            for m in range(M):
                for r in range(NJ):
                    ps = pp.tile([128, TB], F32, tag="pp", name="pp")
                    for c in range(NJ):
                        nc.tensor.matmul(
                            ps,
                            wk_sb[:, c, r * 128:(r + 1) * 128],
                            k_in[m][:, c, :],
                            start=(c == 0),
                            stop=(c == NJ - 1 and not with_bias),
                        )
                    if with_bias:
                        nc.tensor.matmul(
                            ps, bk_sb[:, r * 128:(r + 1) * 128], ones_sb,
                            start=False, stop=True,
                        )
                    nc.scalar.copy(out=kp[r][:, m, :], in_=ps)

            # ---- attention logits, token-major: pslt[t, tt, m, h] ----
            pslt = pl.tile([128, NT, M, H], F32, tag="pl", name="pl")
            for r in range(NJ):
                prod = p_prod.tile([128, M, TB], DT, tag="prod", name="prod")
                nc.vector.tensor_mul(prod, _bcast(qp[r], M, 1), kp[r])
                for tt in range(NT):
                    for m in range(M):
                        nc.tensor.matmul(
                            pslt[:, tt, m, 2 * r:2 * r + 2],
                            prod[:, m, tt * 128:(tt + 1) * 128],
                            indl_sb,
                            start=True,
                            stop=True,
                        )

            # ---- softmax over M (token-major, full 128 partitions) ----
            e = p_sm.tile([128, NT, M, H], F32, tag="e", name="e")
            nc.scalar.activation(e, pslt, func=mybir.ActivationFunctionType.Exp)
            s01 = p_sm.tile([128, NT, H], F32, tag="s01", name="s01")
            s = p_sm.tile([128, NT, H], F32, tag="s", name="s")
            nc.vector.tensor_add(s01, e[:, :, 0, :], e[:, :, 1, :])
            nc.vector.tensor_add(s, e[:, :, 2, :], e[:, :, 3, :])
            nc.vector.tensor_add(s, s01, s)
            rcp = p_sm.tile([128, NT, H], F32, tag="rcp", name="rcp")
            nc.vector.reciprocal(rcp, s)
            w_t = p_sm.tile([128, NT, M, H], DT, tag="w", name="w")
            nc.vector.tensor_mul(w_t, e, _bcast(rcp, M, 2))

            # transpose w to head-major: wT[(m,h), (tt,t)]
            wT = p_sm.tile([64, NT, 128], DT, tag="wT", name="wT")
            for tt in range(NT):
                pst = pw.tile([64, 128], DT, tag="pw", name="pw")
                nc.tensor.transpose(pst, w_t[:, tt, :, :], ident)
                nc.scalar.copy(out=wT[:, tt, :], in_=pst)

            # ---- weighted sum of k^2 (PE broadcast + DVE) ----
            yb = [p_yb.tile([128, TB], DT, tag="yb", name="yb") for _ in range(NJ)]
            for r in range(NJ):
                ksq = p_ksq.tile([128, M, TB], DT, tag="ksq", name="ksq")
                nc.vector.tensor_mul(ksq, kp[r], kp[r])
                ct = p_ct.tile([128, M, TB], F32, tag="ct", name="ct")
                for mp in range(2):      # m-pairs
                    for hf in range(2):  # half-blocks of 256 tokens
                        psb = pb.tile([128, 2, 256], F32, tag="pb", name="pb")
                        for mi in range(2):
                            m = 2 * mp + mi
                            for ti in range(2):
                                tt = 2 * hf + ti
                                nc.tensor.matmul(
                                    psb[:, mi, ti * 128:(ti + 1) * 128],
                                    indb_sb[:, m, r, :],
                                    wT[:, tt, :],
                                    start=True,
                                    stop=True,
                                )
                        nc.vector.tensor_mul(
                            ct[:, 2 * mp:2 * mp + 2, hf * 256:(hf + 1) * 256],
                            psb,
                            ksq[:, 2 * mp:2 * mp + 2, hf * 256:(hf + 1) * 256],
                        )
                y = p_y.tile([128, TB], F32, tag="y", name="y")
                nc.vector.reduce_sum(
                    y, ct.rearrange("p m t -> p t m"), axis=mybir.AxisListType.X
                )
                nc.scalar.copy(out=yb[r], in_=y)  # cast f32 -> fp16

            # ---- output projection (PE) ----
            for tt in range(NT):
                for oc in range(2):
                    ps = pp.tile([128, 512], F32, tag="pp", name="pp")
                    for r in range(NJ):
                        nc.tensor.matmul(
                            ps,
                            yb[r][:, tt * 128:(tt + 1) * 128],
                            wo_sb[:, r, oc * 512:(oc + 1) * 512],
                            start=(r == 0),
                            stop=(r == NJ - 1 and not with_bias),
                        )
                    if with_bias:
                        nc.tensor.matmul(
                            ps,
                            ones_sb[:, :128],
                            bo_sb[:, oc * 512:(oc + 1) * 512],
                            start=False,
                            stop=True,
                        )
                    o_sb = p_out.tile([128, 512], F32, tag="outs", name="osb")
                    nc.scalar.copy(out=o_sb, in_=ps)
                    nc.sync.dma_start(
                        out=out.ap()[t0 + tt * 128:t0 + (tt + 1) * 128,
                                     oc * 512:(oc + 1) * 512],
                        in_=o_sb,
                    )
    nc.compile()
    return nc


def _host_prep(query, key, Wq, Wk, Wo, bq, bk, bo):
    qT = np.ascontiguousarray(query.reshape(T_TOTAL, C).T).astype(NPDT)
    kT = np.ascontiguousarray(key.reshape(T_TOTAL, M, C).transpose(1, 2, 0)).astype(NPDT)

    wqT = np.ascontiguousarray(Wq.T).astype(NPDT)
    wkT = np.ascontiguousarray(Wk.T).astype(NPDT)
    woT = np.ascontiguousarray(Wo.T).astype(NPDT)

    j = np.arange(128)
    indl = np.stack([(j < 64), (j >= 64)], axis=1).astype(NPDT) * NPDT(SCALE)
    # indb[m, r, row, j] = 1 iff row == m*H + 2r + (j >= 64)
    rows = np.arange(64)[None, None, :, None]
    ms = np.arange(M)[:, None, None, None]
    rs = np.arange(NJ)[None, :, None, None]
    indb = (rows == ms * H + 2 * rs + (j[None, None, None, :] >= 64)).astype(NPDT)

    with_bias = bool(np.any(bq) or np.any(bk) or np.any(bo))
    common = {"wqT": wqT, "wkT": wkT, "woT": woT, "indl": indl, "indb": indb}
    if with_bias:
        common |= {
            "bq": bq.reshape(1, C).astype(NPDT),
            "bk": bk.reshape(1, C).astype(NPDT),
            "bo": bo.reshape(1, C).astype(NPDT),
        }
    in_maps = []
    for i in range(NCORES):
        sl = slice(i * T_CORE, (i + 1) * T_CORE)
        in_maps.append(
            {
                "qT": np.ascontiguousarray(qT[:, sl]),
                "kT": np.ascontiguousarray(kT[:, :, sl]),
                **common,
            }
        )
    return in_maps, with_bias


_NC_CACHE = {}
_LAST_RESULT = None


def kernel(query, key, gate, Wq, bq, Wk, bk, Wv, bv, Wg, bg, Wo, bo):
    in_maps, with_bias = _host_prep(query, key, Wq, Wk, Wo, bq, bk, bo)
    key_ = (T_CORE, with_bias)
    if key_ not in _NC_CACHE:
        _NC_CACHE[key_] = build_nc(T_CORE, with_bias)
    nc = _NC_CACHE[key_]
    res = run_bass_kernel_spmd(nc, in_maps, list(range(NCORES)))
    global _LAST_RESULT
    _LAST_RESULT = res
    out = np.concatenate([res.results[i]["out"] for i in range(NCORES)], axis=0)
    return out.reshape(B, N, C)



# revision 2
# speedup vs baseline: 1.3287x; 1.3287x over previous
"""Trainium2 Bass kernel for GatedCrossAttention (B=4, N=4096, C=1024, H=16, M=4).

Reference math (dead code removed: the v/gate projections are overwritten
by views of k in the original module, so v = g = k):
    q = query @ Wq.T + bq                    [B,N,C]   -> [B,N,H,hd]
    k = key   @ Wk.T + bk                    [B,N,M,C] -> [B,N,M,H,hd]
    attn = softmax_M(SCALE * einsum('bnhc,bnmhc->bnmh', q, k))
    out  = einsum('bnmh,bnmhc->bnhc', attn, k*k) . reshape(B,N,C)
    out  = out @ Wo.T + bo

Strategy: pure data parallel over the 16384 tokens (8 cores x 2048), no
collectives.  On-chip layout is "transposed": channels on partitions, tokens
on the free axis, so every matmul contraction (over channels) is a natural
PE op.  The projected channel space is HEAD-INTERLEAVED (new channel
c = d*16 + h, done host-side by permuting Wq/Wk output columns and Wo input
rows): every 128-partition tile then holds all 16 heads, so

  * the per-head logit reduction becomes: sum the 8 per-tile q*k products
    (DVE f16 tree) into one tile R, then 16 tiny indicator matmuls
    (stationary = R token chunk, moving = [128,16] head indicator) produce
    token-major logits for a full-width softmax over M;
  * the softmax-weight head->channel broadcast is ONE [64,128] indicator
    matmul per m (N=512) valid for all 8 channel tiles.

The weighted sum of k^2 is plain f16 DVE mul/add chains (k^2 on the scalar
engine), avoiding the slow TENSOR_REDUCE.  Blocks of 512 tokens are
software-pipelined via deep tile pools so the next block's dense
projection matmuls fill the PE while the current block's attention middle
runs on DVE/ACT — keeping the PE warm (HAM at 2.4GHz) and busy.
Host pre-transposes/casts inputs and weights to fp16 (error vs f32
reference ~1e-3, PE runs 16-bit at full rate), accumulation stays f32.
"""

import dataclasses
import numpy as np
from contextlib import ExitStack

try:
    import concourse.bass as bass
except ImportError:  # path fallback for bare containers
    import sys

    sys.path.insert(0, "/opt/trn_rl_repo")
    import concourse.bass as bass

import concourse.tile as tile
from concourse import bacc, mybir
from concourse.bass_utils import run_bass_kernel_spmd
from concourse.masks import make_identity

# problem constants (hardcoded per the task contract)
B, N, C, H, HD, M = 4, 4096, 1024, 16, 64, 4
SCALE = float(HD) ** -0.5
NCORES = 8
T_TOTAL = B * N
T_CORE = T_TOTAL // NCORES  # 2048
TB = 512                    # tokens per block (one PSUM bank at f32)
NJ = C // 128               # 8 channel tiles
NT = TB // 128              # 4 token subtiles per block

DT = mybir.dt.float16
NPDT = np.float16
F32 = mybir.dt.float32


def _bcast(ap, reps, axis):
    """Insert a 0-stride dim of size `reps` at AP position `axis` (0=partition)."""
    new = list(ap.ap)
    new.insert(axis, [0, reps])
    return dataclasses.replace(ap, ap=new)


def build_nc(t_core=T_CORE, with_bias=False):
    nblk = t_core // TB
    nc = bacc.Bacc("TRN2", target_bir_lowering=False, debug=False)

    qT = nc.declare_dram_parameter("qT", [C, t_core], DT, isOutput=False)
    kT = nc.declare_dram_parameter("kT", [M, C, t_core], DT, isOutput=False)
    wqT = nc.declare_dram_parameter("wqT", [C, C], DT, isOutput=False)
    wkT = nc.declare_dram_parameter("wkT", [C, C], DT, isOutput=False)
    woT = nc.declare_dram_parameter("woT", [C, C], DT, isOutput=False)
    ind16 = nc.declare_dram_parameter("ind16", [128, H], DT, isOutput=False)
    indw = nc.declare_dram_parameter("indw", [64, M, 128], DT, isOutput=False)
    if with_bias:
        bq = nc.declare_dram_parameter("bq", [1, C], DT, isOutput=False)
        bk = nc.declare_dram_parameter("bk", [1, C], DT, isOutput=False)
        bo = nc.declare_dram_parameter("bo", [1, C], DT, isOutput=False)
    out = nc.declare_dram_parameter("out", [t_core, C], F32, isOutput=True)

    # DRAM views: channel dim split into (chunk, partition)
    qT_v = qT.ap().rearrange("(c p) t -> p c t", p=128)
    kT_v = kT.ap().rearrange("m (c p) t -> p m c t", p=128)
    wq_v = wqT.ap().rearrange("(c p) j -> p c j", p=128)
    wk_v = wkT.ap().rearrange("(c p) j -> p c j", p=128)
    wo_v = woT.ap().rearrange("(c p) j -> p c j", p=128)

    with tile.TileContext(nc) as tc, ExitStack() as ctx:
        consts = ctx.enter_context(tc.tile_pool(name="consts", bufs=1))
        p_inq = ctx.enter_context(tc.tile_pool(name="inq", bufs=2))
        p_ink = ctx.enter_context(tc.tile_pool(name="ink", bufs=4))
        p_qp = ctx.enter_context(tc.tile_pool(name="qp", bufs=9))
        p_kp = ctx.enter_context(tc.tile_pool(name="kp", bufs=11))
        p_R = ctx.enter_context(tc.tile_pool(name="R", bufs=2))
        p_tmp = ctx.enter_context(tc.tile_pool(name="tmp", bufs=2))
        p_ksq = ctx.enter_context(tc.tile_pool(name="ksq", bufs=2))
        p_sm = ctx.enter_context(tc.tile_pool(name="sm", bufs=2))
        p_wT = ctx.enter_context(tc.tile_pool(name="wT", bufs=2))
        p_wbc = ctx.enter_context(tc.tile_pool(name="wbc", bufs=4))
        p_ct = ctx.enter_context(tc.tile_pool(name="ct", bufs=2))
        p_yb = ctx.enter_context(tc.tile_pool(name="yb", bufs=10))
        p_out = ctx.enter_context(tc.tile_pool(name="outs", bufs=3))
        pp = ctx.enter_context(tc.tile_pool(name="pp", bufs=2, space="PSUM"))
        pl = ctx.enter_context(tc.tile_pool(name="pl", bufs=2, space="PSUM"))
        pw = ctx.enter_context(tc.tile_pool(name="pw", bufs=1, space="PSUM"))
        pb = ctx.enter_context(tc.tile_pool(name="pb", bufs=2, space="PSUM"))

        # ---- constants / weights (resident) ----
        wq_sb = consts.tile([128, NJ, C], DT)
        wk_sb = consts.tile([128, NJ, C], DT)
        wo_sb = consts.tile([128, NJ, C], DT)
        nc.sync.dma_start(out=wq_sb, in_=wq_v)
        nc.sync.dma_start(out=wk_sb, in_=wk_v)
        nc.sync.dma_start(out=wo_sb, in_=wo_v)
        ind16_sb = consts.tile([128, H], DT)
        indw_sb = consts.tile([64, M, 128], DT)
        nc.sync.dma_start(out=ind16_sb, in_=ind16.ap())
        nc.sync.dma_start(out=indw_sb, in_=indw.ap())
        ident = consts.tile([128, 128], DT)
        make_identity(nc, ident)
        if with_bias:
            ones_sb = consts.tile([1, TB], DT)
            nc.vector.memset(ones_sb, 1.0)
            bq_sb = consts.tile([1, C], DT)
            bk_sb = consts.tile([1, C], DT)
            bo_sb = consts.tile([1, C], DT)
            nc.sync.dma_start(out=bq_sb, in_=bq.ap())
            nc.sync.dma_start(out=bk_sb, in_=bk.ap())
            nc.sync.dma_start(out=bo_sb, in_=bo.ap())

        for blk in range(nblk):
            t0 = blk * TB
            tsl = slice(t0, t0 + TB)

            # ---- load inputs ----
            q_in = p_inq.tile([128, NJ, TB], DT)
            nc.sync.dma_start(out=q_in, in_=qT_v[:, :, tsl])
            k_in = [p_ink.tile([128, NJ, TB], DT, tag="kin", name="kin")
                    for _ in range(M)]
            for m in range(M):
                nc.sync.dma_start(out=k_in[m], in_=kT_v[:, m, :, tsl])

            # ---- Q projection (PE) ----
            qp = [p_qp.tile([128, TB], DT, tag="qp", name="qp") for _ in range(NJ)]
            for r in range(NJ):
                ps = pp.tile([128, TB], F32, tag="pp", name="pp")
                for c in range(NJ):
                    nc.tensor.matmul(
                        ps,
                        wq_sb[:, c, r * 128:(r + 1) * 128],
                        q_in[:, c, :],
                        start=(c == 0),
                        stop=(c == NJ - 1 and not with_bias),
                    )
                if with_bias:
                    nc.tensor.matmul(
                        ps, bq_sb[:, r * 128:(r + 1) * 128], ones_sb,
                        start=False, stop=True,
                    )
                nc.scalar.copy(out=qp[r], in_=ps)

            # ---- K projection (PE), r-outer so early r tiles finish first ----
            kp = [p_kp.tile([128, M, TB], DT, tag="kp", name="kp")
                  for _ in range(NJ)]
            for r in range(NJ):
                for m in range(M):
                    ps = pp.tile([128, TB], F32, tag="pp", name="pp")
                    for c in range(NJ):
                        nc.tensor.matmul(
                            ps,
                            wk_sb[:, c, r * 128:(r + 1) * 128],
                            k_in[m][:, c, :],
                            start=(c == 0),
                            stop=(c == NJ - 1 and not with_bias),
                        )
                    if with_bias:
                        nc.tensor.matmul(
                            ps, bk_sb[:, r * 128:(r + 1) * 128], ones_sb,
                            start=False, stop=True,
                        )
                    nc.scalar.copy(out=kp[r][:, m, :], in_=ps)

            # ---- R = sum_r qp_r * kp_r  (DVE f16; heads interleaved) ----
            R = p_R.tile([128, M, TB], DT, tag="R", name="R")
            nc.vector.tensor_mul(R, _bcast(qp[0], M, 1), kp[0])
            for r in range(1, NJ):
                t = p_tmp.tile([128, M, TB], DT, tag="tmp", name="tmp")
                nc.vector.tensor_mul(t, _bcast(qp[r], M, 1), kp[r])
                nc.vector.tensor_add(R, R, t)

            # ---- attention logits, token-major: pslt[t, tt, m, h] ----
            pslt = pl.tile([128, NT, M, H], F32, tag="pl", name="pl")
            for m in range(M):
                for tt in range(NT):
                    nc.tensor.matmul(
                        pslt[:, tt, m, :],
                        R[:, m, tt * 128:(tt + 1) * 128],
                        ind16_sb,
                        start=True,
                        stop=True,
                    )

            # ---- softmax over M (token-major, full 128 partitions) ----
            e = p_sm.tile([128, NT, M, H], F32, tag="e", name="e")
            nc.scalar.activation(e, pslt, func=mybir.ActivationFunctionType.Exp)
            s01 = p_sm.tile([128, NT, H], F32, tag="s01", name="s01")
            s = p_sm.tile([128, NT, H], F32, tag="s", name="s")
            nc.vector.tensor_add(s01, e[:, :, 0, :], e[:, :, 1, :])
            nc.vector.tensor_add(s, e[:, :, 2, :], e[:, :, 3, :])
            nc.vector.tensor_add(s, s01, s)
            rcp = p_sm.tile([128, NT, H], F32, tag="rcp", name="rcp")
            nc.vector.reciprocal(rcp, s)
            w_t = p_sm.tile([128, NT, M, H], DT, tag="w", name="w")
            nc.vector.tensor_mul(w_t, e, _bcast(rcp, M, 2))

            # transpose w to head-major: wT[(m,h), (tt,t)]
            wT = p_wT.tile([64, NT, 128], DT, tag="wT", name="wT")
            for tt in range(NT):
                pst = pw.tile([64, 128], DT, tag="pw", name="pw")
                nc.tensor.transpose(pst, w_t[:, tt, :, :], ident)
                nc.scalar.copy(out=wT[:, tt, :], in_=pst)

            # ---- head->channel broadcast of softmax weights (PE, one MM per m) ----
            wbc = [p_wbc.tile([128, TB], DT, tag="wbc", name="wbc")
                   for _ in range(M)]
            for m in range(M):
                psb = pb.tile([128, TB], F32, tag="pb", name="pb")
                nc.tensor.matmul(
                    psb, indw_sb[:, m, :], wT, start=True, stop=True,
                )
                nc.scalar.copy(out=wbc[m], in_=psb)

            # ---- weighted sum of k^2 (ACT square + DVE mul/add chains) ----
            yb = [p_yb.tile([128, TB], DT, tag="yb", name="yb") for _ in range(NJ)]
            for r in range(NJ):
                ksq = p_ksq.tile([128, M, TB], DT, tag="ksq", name="ksq")
                nc.scalar.activation(
                    ksq, kp[r], func=mybir.ActivationFunctionType.Square
                )
                nc.vector.tensor_mul(yb[r], wbc[0], ksq[:, 0, :])
                for m in range(1, M):
                    ct = p_ct.tile([128, TB], DT, tag="ct", name="ct")
                    nc.vector.tensor_mul(ct, wbc[m], ksq[:, m, :])
                    nc.vector.tensor_add(yb[r], yb[r], ct)

            # ---- output projection (PE) ----
            for tt in range(NT):
                for oc in range(2):
                    ps = pp.tile([128, 512], F32, tag="pp", name="pp")
                    for r in range(NJ):
                        nc.tensor.matmul(
                            ps,
                            yb[r][:, tt * 128:(tt + 1) * 128],
                            wo_sb[:, r, oc * 512:(oc + 1) * 512],
                            start=(r == 0),
                            stop=(r == NJ - 1 and not with_bias),
                        )
                    if with_bias:
                        nc.tensor.matmul(
                            ps,
                            ones_sb[:, :128],
                            bo_sb[:, oc * 512:(oc + 1) * 512],
                            start=False,
                            stop=True,
                        )
                    o_sb = p_out.tile([128, 512], F32, tag="outs", name="osb")
                    nc.scalar.copy(out=o_sb, in_=ps)
                    nc.sync.dma_start(
                        out=out.ap()[t0 + tt * 128:t0 + (tt + 1) * 128,
                                     oc * 512:(oc + 1) * 512],
                        in_=o_sb,
                    )
    nc.compile()
    return nc


def _host_prep(query, key, Wq, Wk, Wo, bq, bk, bo):
    qT = np.ascontiguousarray(query.reshape(T_TOTAL, C).T).astype(NPDT)
    kT = np.ascontiguousarray(key.reshape(T_TOTAL, M, C).transpose(1, 2, 0)).astype(NPDT)

    # head-interleaved projection space: new channel c = d*16 + h
    cc = np.arange(C)
    old0 = (cc % H) * HD + cc // H
    wqT = np.ascontiguousarray(Wq.T[:, old0]).astype(NPDT)
    wkT = np.ascontiguousarray(Wk.T[:, old0]).astype(NPDT)
    woT = np.ascontiguousarray(Wo.T[old0, :]).astype(NPDT)

    p = np.arange(128)
    ind16 = (p[:, None] % H == np.arange(H)[None, :]).astype(NPDT) * NPDT(SCALE)
    # indw[q, m, p] = 1 iff q == m*H + (p % H)
    q_ = np.arange(64)[:, None, None]
    m_ = np.arange(M)[None, :, None]
    indw = (q_ == m_ * H + p[None, None, :] % H).astype(NPDT)

    with_bias = bool(np.any(bq) or np.any(bk) or np.any(bo))
    common = {"wqT": wqT, "wkT": wkT, "woT": woT, "ind16": ind16, "indw": indw}
    if with_bias:
        common |= {
            "bq": bq.reshape(1, C)[:, old0].astype(NPDT),
            "bk": bk.reshape(1, C)[:, old0].astype(NPDT),
            "bo": bo.reshape(1, C).astype(NPDT),
        }
    in_maps = []
    for i in range(NCORES):
        sl = slice(i * T_CORE, (i + 1) * T_CORE)
        in_maps.append(
            {
                "qT": np.ascontiguousarray(qT[:, sl]),
                "kT": np.ascontiguousarray(kT[:, :, sl]),
                **common,
            }
        )
    return in_maps, with_bias


_NC_CACHE = {}
_LAST_RESULT = None


def kernel(query, key, gate, Wq, bq, Wk, bk, Wv, bv, Wg, bg, Wo, bo):
    in_maps, with_bias = _host_prep(query, key, Wq, Wk, Wo, bq, bk, bo)
    key_ = (T_CORE, with_bias)
    if key_ not in _NC_CACHE:
        _NC_CACHE[key_] = build_nc(T_CORE, with_bias)
    nc = _NC_CACHE[key_]
    res = run_bass_kernel_spmd(nc, in_maps, list(range(NCORES)))
    global _LAST_RESULT
    _LAST_RESULT = res
    out = np.concatenate([res.results[i]["out"] for i in range(NCORES)], axis=0)
    return out.reshape(B, N, C)


# revision 5
# speedup vs baseline: 1.3490x; 1.0152x over previous
"""Trainium2 Bass kernel for GatedCrossAttention (B=4, N=4096, C=1024, H=16, M=4).

Reference math (dead code removed: the v/gate projections are overwritten
by views of k in the original module, so v = g = k):
    q = query @ Wq.T + bq                    [B,N,C]   -> [B,N,H,hd]
    k = key   @ Wk.T + bk                    [B,N,M,C] -> [B,N,M,H,hd]
    attn = softmax_M(SCALE * einsum('bnhc,bnmhc->bnmh', q, k))
    out  = einsum('bnmh,bnmhc->bnhc', attn, k*k) . reshape(B,N,C)
    out  = out @ Wo.T + bo

Strategy: pure data parallel over the 16384 tokens (8 cores x 2048), no
collectives.  On-chip layout is "transposed": channels on partitions, tokens
on the free axis, so every matmul contraction (over channels) is a natural
PE op.  The projected channel space is HEAD-INTERLEAVED (new channel
c = d*16 + h, done host-side by permuting Wq/Wk output columns and Wo input
rows): every 128-partition tile then holds all 16 heads, so

  * the per-head logit reduction becomes: sum the 8 per-tile q*k products
    (DVE f16 chains, one accumulator per m) into R_m, then 16 tiny
    indicator matmuls (stationary = R_m token chunk, moving = [128,16]
    head indicator) produce token-major logits for a full-width softmax;
  * the softmax-weight head->channel broadcast is ONE [64,128] indicator
    matmul per m (N=512) valid for all 8 channel tiles.

k^2 runs on the (otherwise idle) GpSimd engine; the weighted sum is plain
f16 DVE mul/add chains.  The K projection is m-outer with per-(r,m) kp
tiles so products/R_m accumulate DURING the projection stream, and blocks
of 512 tokens are software-pipelined via deep tile pools.  The output
projection has its own PSUM pool so its (slow, yb-gated) accumulation
groups never block the next block's projection matmuls — the PE stays
dense and warm (HAM at 2.4GHz).  Host pre-transposes/casts inputs and
weights to fp16 (error vs f32 reference ~1e-3, PE runs 16-bit at full
rate), accumulation stays f32.
"""

import dataclasses
import numpy as np
from contextlib import ExitStack

try:
    import concourse.bass as bass
except ImportError:  # path fallback for bare containers
    import sys

    sys.path.insert(0, "/opt/trn_rl_repo")
    import concourse.bass as bass

import concourse.tile as tile
from concourse import bacc, mybir
from concourse.bass_utils import run_bass_kernel_spmd
from concourse.masks import make_identity

# problem constants (hardcoded per the task contract)
B, N, C, H, HD, M = 4, 4096, 1024, 16, 64, 4
SCALE = float(HD) ** -0.5
NCORES = 8
T_TOTAL = B * N
T_CORE = T_TOTAL // NCORES  # 2048
TB = 512                    # tokens per block (one PSUM bank at f32)
NJ = C // 128               # 8 channel tiles
NT = TB // 128              # 4 token subtiles per block

DT = mybir.dt.float16
NPDT = np.float16
F32 = mybir.dt.float32


def _bcast(ap, reps, axis):
    """Insert a 0-stride dim of size `reps` at AP position `axis` (0=partition)."""
    new = list(ap.ap)
    new.insert(axis, [0, reps])
    return dataclasses.replace(ap, ap=new)


def build_nc(t_core=T_CORE, with_bias=False):
    nblk = t_core // TB
    nc = bacc.Bacc("TRN2", target_bir_lowering=False, debug=False)

    qT = nc.declare_dram_parameter("qT", [C, t_core], DT, isOutput=False)
    kT = nc.declare_dram_parameter("kT", [M, C, t_core], DT, isOutput=False)
    wqT = nc.declare_dram_parameter("wqT", [C, C], DT, isOutput=False)
    wkT = nc.declare_dram_parameter("wkT", [C, C], DT, isOutput=False)
    woT = nc.declare_dram_parameter("woT", [C, C], DT, isOutput=False)
    ind16 = nc.declare_dram_parameter("ind16", [128, H], DT, isOutput=False)
    indw = nc.declare_dram_parameter("indw", [64, M, 128], DT, isOutput=False)
    if with_bias:
        bq = nc.declare_dram_parameter("bq", [1, C], DT, isOutput=False)
        bk = nc.declare_dram_parameter("bk", [1, C], DT, isOutput=False)
        bo = nc.declare_dram_parameter("bo", [1, C], DT, isOutput=False)
    out = nc.declare_dram_parameter("out", [t_core, C], F32, isOutput=True)

    # DRAM views: channel dim split into (chunk, partition)
    qT_v = qT.ap().rearrange("(c p) t -> p c t", p=128)
    kT_v = kT.ap().rearrange("m (c p) t -> p m c t", p=128)
    wq_v = wqT.ap().rearrange("(c p) j -> p c j", p=128)
    wk_v = wkT.ap().rearrange("(c p) j -> p c j", p=128)
    wo_v = woT.ap().rearrange("(c p) j -> p c j", p=128)

    with tile.TileContext(nc) as tc, ExitStack() as ctx:
        consts = ctx.enter_context(tc.tile_pool(name="consts", bufs=1))
        p_inq = ctx.enter_context(tc.tile_pool(name="inq", bufs=2))
        p_ink = ctx.enter_context(tc.tile_pool(name="ink", bufs=3))
        p_qp = ctx.enter_context(tc.tile_pool(name="qp", bufs=9))
        p_kp = ctx.enter_context(tc.tile_pool(name="kp", bufs=56))
        p_R = ctx.enter_context(tc.tile_pool(name="R", bufs=8))
        p_tmp = ctx.enter_context(tc.tile_pool(name="tmp", bufs=2))
        p_ksq = ctx.enter_context(tc.tile_pool(name="ksq", bufs=8))
        p_sm = ctx.enter_context(tc.tile_pool(name="sm", bufs=2))
        p_wT = ctx.enter_context(tc.tile_pool(name="wT", bufs=2))
        p_wbc = ctx.enter_context(tc.tile_pool(name="wbc", bufs=8))
        p_ct = ctx.enter_context(tc.tile_pool(name="ct", bufs=2))
        p_yb = ctx.enter_context(tc.tile_pool(name="yb", bufs=10))
        p_out = ctx.enter_context(tc.tile_pool(name="outs", bufs=3))
        pp = ctx.enter_context(tc.tile_pool(name="pp", bufs=2, space="PSUM"))
        po = ctx.enter_context(tc.tile_pool(name="po", bufs=2, space="PSUM"))
        pl = ctx.enter_context(tc.tile_pool(name="pl", bufs=1, space="PSUM"))
        pw = ctx.enter_context(tc.tile_pool(name="pw", bufs=2, space="PSUM"))
        pb = ctx.enter_context(tc.tile_pool(name="pb", bufs=1, space="PSUM"))

        # ---- constants / weights (resident) ----
        wq_sb = consts.tile([128, NJ, C], DT)
        wk_sb = consts.tile([128, NJ, C], DT)
        nc.sync.dma_start(out=wq_sb, in_=wq_v)
        nc.sync.dma_start(out=wk_sb, in_=wk_v)
        ind16_sb = consts.tile([128, H], DT)
        indw_sb = consts.tile([64, M, 128], DT)
        nc.sync.dma_start(out=ind16_sb, in_=ind16.ap())
        nc.sync.dma_start(out=indw_sb, in_=indw.ap())
        wo_sb = consts.tile([128, NJ, C], DT)
        nc.sync.dma_start(out=wo_sb, in_=wo_v)
        ident = consts.tile([128, 128], DT)
        make_identity(nc, ident)
        if with_bias:
            ones_sb = consts.tile([1, TB], DT)
            nc.vector.memset(ones_sb, 1.0)
            bq_sb = consts.tile([1, C], DT)
            bk_sb = consts.tile([1, C], DT)
            bo_sb = consts.tile([1, C], DT)
            nc.sync.dma_start(out=bq_sb, in_=bq.ap())
            nc.sync.dma_start(out=bk_sb, in_=bk.ap())
            nc.sync.dma_start(out=bo_sb, in_=bo.ap())

        for blk in range(nblk):
            t0 = blk * TB
            tsl = slice(t0, t0 + TB)

            # ---- load inputs ----
            q_in = p_inq.tile([128, NJ, TB], DT)
            nc.sync.dma_start(out=q_in, in_=qT_v[:, :, tsl])
            k_in = [p_ink.tile([128, NJ, TB], DT, tag="kin", name="kin")
                    for _ in range(M)]
            for m in range(M):
                nc.sync.dma_start(out=k_in[m], in_=kT_v[:, m, :, tsl])

            # ---- Q projection (PE) ----
            qp = [p_qp.tile([128, TB], DT, tag="qp", name="qp") for _ in range(NJ)]
            for r in range(NJ):
                ps = pp.tile([128, TB], F32, tag="pp", name="pp")
                for c in range(NJ):
                    nc.tensor.matmul(
                        ps,
                        wq_sb[:, c, r * 128:(r + 1) * 128],
                        q_in[:, c, :],
                        start=(c == 0),
                        stop=(c == NJ - 1 and not with_bias),
                    )
                if with_bias:
                    nc.tensor.matmul(
                        ps, bq_sb[:, r * 128:(r + 1) * 128], ones_sb,
                        start=False, stop=True,
                    )
                nc.scalar.copy(out=qp[r], in_=ps)

            # ---- K projection (PE), m-outer; per-(r,m) kp tiles so the
            #      q*k product chains accumulate during the stream ----
            kp = [[None] * M for _ in range(NJ)]
            Rs = []
            for m in range(M):
                for r in range(NJ):
                    ps = pp.tile([128, TB], F32, tag="pp", name="pp")
                    for c in range(NJ):
                        nc.tensor.matmul(
                            ps,
                            wk_sb[:, c, r * 128:(r + 1) * 128],
                            k_in[m][:, c, :],
                            start=(c == 0),
                            stop=(c == NJ - 1 and not with_bias),
                        )
                    if with_bias:
                        nc.tensor.matmul(
                            ps, bk_sb[:, r * 128:(r + 1) * 128], ones_sb,
                            start=False, stop=True,
                        )
                    kp[r][m] = p_kp.tile([128, TB], DT, tag="kp", name="kp")
                    nc.scalar.copy(out=kp[r][m], in_=ps)

                # R_m = sum_r qp_r * kp_r_m  (DVE f16, fires as copies land)
                Rm = p_R.tile([128, TB], DT, tag="R", name="R")
                nc.vector.tensor_mul(Rm, qp[0], kp[0][m])
                for r in range(1, NJ):
                    t = p_tmp.tile([128, TB], DT, tag="tmp", name="tmp")
                    nc.vector.tensor_mul(t, qp[r], kp[r][m])
                    nc.vector.tensor_add(Rm, Rm, t)
                Rs.append(Rm)

            # ---- attention logits, token-major: pslt[t, tt, m, h] ----
            pslt = pl.tile([128, NT, M, H], F32, tag="pl", name="pl")
            for m in range(M):
                for tt in range(NT):
                    nc.tensor.matmul(
                        pslt[:, tt, m, :],
                        Rs[m][:, tt * 128:(tt + 1) * 128],
                        ind16_sb,
                        start=True,
                        stop=True,
                    )

            # ---- softmax over M (token-major, full 128 partitions) ----
            e = p_sm.tile([128, NT, M, H], F32, tag="e", name="e")
            nc.scalar.activation(e, pslt, func=mybir.ActivationFunctionType.Exp)
            s01 = p_sm.tile([128, NT, H], F32, tag="s01", name="s01")
            s = p_sm.tile([128, NT, H], F32, tag="s", name="s")
            nc.vector.tensor_add(s01, e[:, :, 0, :], e[:, :, 1, :])
            nc.vector.tensor_add(s, e[:, :, 2, :], e[:, :, 3, :])
            nc.vector.tensor_add(s, s01, s)
            rcp = p_sm.tile([128, NT, H], F32, tag="rcp", name="rcp")
            nc.vector.reciprocal(rcp, s)
            w_t = p_sm.tile([128, NT, M, H], DT, tag="w", name="w")
            nc.vector.tensor_mul(w_t, e, _bcast(rcp, M, 2))

            # transpose w to head-major: wT[(m,h), (tt,t)]
            wT = p_wT.tile([64, NT, 128], DT, tag="wT", name="wT")
            for tt in range(NT):
                pst = pw.tile([64, 128], DT, tag="pw", name="pw")
                nc.tensor.transpose(pst, w_t[:, tt, :, :], ident)
                nc.scalar.copy(out=wT[:, tt, :], in_=pst)

            # ---- head->channel broadcast of softmax weights (PE, one MM per m) ----
            wbc = [p_wbc.tile([128, TB], DT, tag="wbc", name="wbc")
                   for _ in range(M)]
            for m in range(M):
                psb = pb.tile([128, TB], F32, tag="pb", name="pb")
                nc.tensor.matmul(
                    psb, indw_sb[:, m, :], wT, start=True, stop=True,
                )
                nc.scalar.copy(out=wbc[m], in_=psb)

            # ---- weighted sum of k^2 (GpSimd square + DVE mul/add chains) ----
            yb = [p_yb.tile([128, TB], DT, tag="yb", name="yb") for _ in range(NJ)]
            for r in range(NJ):
                ksq = [p_ksq.tile([128, TB], DT, tag="ksq", name="ksq")
                       for _ in range(M)]
                for m in range(M):
                    nc.gpsimd.tensor_mul(ksq[m], kp[r][m], kp[r][m])
                nc.vector.tensor_mul(yb[r], wbc[0], ksq[0])
                for m in range(1, M):
                    ct = p_ct.tile([128, TB], DT, tag="ct", name="ct")
                    nc.vector.tensor_mul(ct, wbc[m], ksq[m])
                    nc.vector.tensor_add(yb[r], yb[r], ct)

            # ---- output projection (PE, own PSUM pool so the yb-gated
            #      accumulation groups never starve the projection stream) ----
            for tt in range(NT):
                for oc in range(2):
                    ps = po.tile([128, 512], F32, tag="po", name="po")
                    for r in range(NJ):
                        nc.tensor.matmul(
                            ps,
                            yb[r][:, tt * 128:(tt + 1) * 128],
                            wo_sb[:, r, oc * 512:(oc + 1) * 512],
                            start=(r == 0),
                            stop=(r == NJ - 1 and not with_bias),
                        )
                    if with_bias:
                        nc.tensor.matmul(
                            ps,
                            ones_sb[:, :128],
                            bo_sb[:, oc * 512:(oc + 1) * 512],
                            start=False,
                            stop=True,
                        )
                    o_sb = p_out.tile([128, 512], F32, tag="outs", name="osb")
                    nc.scalar.copy(out=o_sb, in_=ps)
                    nc.sync.dma_start(
                        out=out.ap()[t0 + tt * 128:t0 + (tt + 1) * 128,
                                     oc * 512:(oc + 1) * 512],
                        in_=o_sb,
                    )
    nc.compile()
    return nc


def _host_prep(query, key, Wq, Wk, Wo, bq, bk, bo):
    qT = np.ascontiguousarray(query.reshape(T_TOTAL, C).T).astype(NPDT)
    kT = np.ascontiguousarray(key.reshape(T_TOTAL, M, C).transpose(1, 2, 0)).astype(NPDT)

    # head-interleaved projection space: new channel c = d*16 + h
    cc = np.arange(C)
    old0 = (cc % H) * HD + cc // H
    wqT = np.ascontiguousarray(Wq.T[:, old0]).astype(NPDT)
    wkT = np.ascontiguousarray(Wk.T[:, old0]).astype(NPDT)
    woT = np.ascontiguousarray(Wo.T[old0, :]).astype(NPDT)

    p = np.arange(128)
    ind16 = (p[:, None] % H == np.arange(H)[None, :]).astype(NPDT) * NPDT(SCALE)
    # indw[q, m, p] = 1 iff q == m*H + (p % H)
    q_ = np.arange(64)[:, None, None]
    m_ = np.arange(M)[None, :, None]
    indw = (q_ == m_ * H + p[None, None, :] % H).astype(NPDT)

    with_bias = bool(np.any(bq) or np.any(bk) or np.any(bo))
    common = {"wqT": wqT, "wkT": wkT, "woT": woT, "ind16": ind16, "indw": indw}
    if with_bias:
        common |= {
            "bq": bq.reshape(1, C)[:, old0].astype(NPDT),
            "bk": bk.reshape(1, C)[:, old0].astype(NPDT),
            "bo": bo.reshape(1, C).astype(NPDT),
        }
    in_maps = []
    for i in range(NCORES):
        sl = slice(i * T_CORE, (i + 1) * T_CORE)
        in_maps.append(
            {
                "qT": np.ascontiguousarray(qT[:, sl]),
                "kT": np.ascontiguousarray(kT[:, :, sl]),
                **common,
            }
        )
    return in_maps, with_bias


_NC_CACHE = {}
_LAST_RESULT = None


def kernel(query, key, gate, Wq, bq, Wk, bk, Wv, bv, Wg, bg, Wo, bo):
    in_maps, with_bias = _host_prep(query, key, Wq, Wk, Wo, bq, bk, bo)
    key_ = (T_CORE, with_bias)
    if key_ not in _NC_CACHE:
        _NC_CACHE[key_] = build_nc(T_CORE, with_bias)
    nc = _NC_CACHE[key_]
    res = run_bass_kernel_spmd(nc, in_maps, list(range(NCORES)))
    global _LAST_RESULT
    _LAST_RESULT = res
    out = np.concatenate([res.results[i]["out"] for i in range(NCORES)], axis=0)
    return out.reshape(B, N, C)


# revision 6
# speedup vs baseline: 1.6213x; 1.2019x over previous
"""Trainium2 Bass kernel for GatedCrossAttention (B=4, N=4096, C=1024, H=16, M=4).

Reference math (dead code removed: the v/gate projections are overwritten
by views of k in the original module, so v = g = k):
    q = query @ Wq.T + bq                    [B,N,C]   -> [B,N,H,hd]
    k = key   @ Wk.T + bk                    [B,N,M,C] -> [B,N,M,H,hd]
    attn = softmax_M(SCALE * einsum('bnhc,bnmhc->bnmh', q, k))
    out  = einsum('bnmh,bnmhc->bnhc', attn, k*k) . reshape(B,N,C)
    out  = out @ Wo.T + bo

Strategy: pure data parallel over the 16384 tokens (8 cores x 2048), no
collectives.  On-chip layout is "transposed": channels on partitions, tokens
on the free axis, so every matmul contraction (over channels) is a natural
PE op.  The projected channel space is HEAD-INTERLEAVED (new channel
c = d*16 + h, done host-side by permuting Wq/Wk output columns and Wo input
rows): every 128-partition tile then holds all 16 heads, so

  * the per-head logit reduction becomes: sum the 8 per-tile q*k products
    (DVE f16 chains, one accumulator per m-pair) into R, then 16 tiny
    indicator matmuls (stationary = R token chunk, moving = [128,16]
    head indicator) produce token-major logits for a full-width softmax;
  * the softmax-weight head->channel broadcast is ONE [64,128] indicator
    matmul per m (N=512) valid for all 8 channel tiles.

k^2 runs on the scalar engine (NOT GpSimd: GpSimd shares DVE's second
SBUF port pair and serializes against it); the weighted sum is f16 DVE
mul/add chains over m-PAIR tiles (half the instruction count).  The K
projection is m-outer with per-(r,m-pair) kp tiles so the q*k chains
accumulate DURING the projection stream.  Blocks of 512 tokens are
software-pipelined via deep tile pools, and the output projection has its
own PSUM pool so its (slow, yb-gated) accumulation groups never block the
next block's projection matmuls — the PE stays dense and warm (HAM at
2.4GHz).  Startup DMAs are ordered first-need-first with the Wq/Wk loads
split per output chunk, so the first matmul fires within a few us.  Host
pre-transposes/casts inputs and weights to fp16 (error vs f32 reference
~1e-3), accumulation stays f32.
"""

import dataclasses
import numpy as np
from contextlib import ExitStack

try:
    import concourse.bass as bass
except ImportError:  # path fallback for bare containers
    import sys

    sys.path.insert(0, "/opt/trn_rl_repo")
    import concourse.bass as bass

import concourse.tile as tile
from concourse import bacc, mybir
from concourse.bass_utils import run_bass_kernel_spmd
from concourse.masks import make_identity

# problem constants (hardcoded per the task contract)
B, N, C, H, HD, M = 4, 4096, 1024, 16, 64, 4
SCALE = float(HD) ** -0.5
NCORES = 8
T_TOTAL = B * N
T_CORE = T_TOTAL // NCORES  # 2048
TB = 512                    # tokens per block (one PSUM bank at f32)
NJ = C // 128               # 8 channel tiles
NT = TB // 128              # 4 token subtiles per block
MP = M // 2                 # m-pairs

DT = mybir.dt.float16
NPDT = np.float16
F32 = mybir.dt.float32


def _bcast(ap, reps, axis):
    """Insert a 0-stride dim of size `reps` at AP position `axis` (0=partition)."""
    new = list(ap.ap)
    new.insert(axis, [0, reps])
    return dataclasses.replace(ap, ap=new)


def build_nc(t_core=T_CORE, with_bias=False):
    nblk = t_core // TB
    nc = bacc.Bacc("TRN2", target_bir_lowering=False, debug=False)

    qT = nc.declare_dram_parameter("qT", [C, t_core], DT, isOutput=False)
    kT = nc.declare_dram_parameter("kT", [M, C, t_core], DT, isOutput=False)
    wqT = nc.declare_dram_parameter("wqT", [C, C], DT, isOutput=False)
    wkT = nc.declare_dram_parameter("wkT", [C, C], DT, isOutput=False)
    woT = nc.declare_dram_parameter("woT", [C, C], DT, isOutput=False)
    ind16 = nc.declare_dram_parameter("ind16", [128, H], DT, isOutput=False)
    indw = nc.declare_dram_parameter("indw", [64, M, 128], DT, isOutput=False)
    if with_bias:
        bq = nc.declare_dram_parameter("bq", [1, C], DT, isOutput=False)
        bk = nc.declare_dram_parameter("bk", [1, C], DT, isOutput=False)
        bo = nc.declare_dram_parameter("bo", [1, C], DT, isOutput=False)
    out = nc.declare_dram_parameter("out", [t_core, C], F32, isOutput=True)

    # DRAM views: channel dim split into (chunk, partition)
    qT_v = qT.ap().rearrange("(c p) t -> p c t", p=128)
    kT_v = kT.ap().rearrange("m (c p) t -> p m c t", p=128)
    wq_v = wqT.ap().rearrange("(c p) j -> p c j", p=128)
    wk_v = wkT.ap().rearrange("(c p) j -> p c j", p=128)
    wo_v = woT.ap().rearrange("(c p) j -> p c j", p=128)

    with tile.TileContext(nc) as tc, ExitStack() as ctx:
        consts = ctx.enter_context(tc.tile_pool(name="consts", bufs=1))
        p_inq = ctx.enter_context(tc.tile_pool(name="inq", bufs=2))
        p_ink = ctx.enter_context(tc.tile_pool(name="ink", bufs=3))
        p_qp = ctx.enter_context(tc.tile_pool(name="qp", bufs=8))
        p_kp = ctx.enter_context(tc.tile_pool(name="kp", bufs=28))
        p_R = ctx.enter_context(tc.tile_pool(name="R", bufs=3))
        p_tmp = ctx.enter_context(tc.tile_pool(name="tmp", bufs=2))
        p_ksq = ctx.enter_context(tc.tile_pool(name="ksq", bufs=4))
        p_sm = ctx.enter_context(tc.tile_pool(name="sm", bufs=2))
        p_wT = ctx.enter_context(tc.tile_pool(name="wT", bufs=2))
        p_wbc = ctx.enter_context(tc.tile_pool(name="wbc", bufs=2))
        p_ct = ctx.enter_context(tc.tile_pool(name="ct", bufs=2))
        p_yb = ctx.enter_context(tc.tile_pool(name="yb", bufs=9))
        p_out = ctx.enter_context(tc.tile_pool(name="outs", bufs=3))
        pp = ctx.enter_context(tc.tile_pool(name="pp", bufs=2, space="PSUM"))
        po = ctx.enter_context(tc.tile_pool(name="po", bufs=2, space="PSUM"))
        pl = ctx.enter_context(tc.tile_pool(name="pl", bufs=1, space="PSUM"))
        pw = ctx.enter_context(tc.tile_pool(name="pw", bufs=2, space="PSUM"))
        pb = ctx.enter_context(tc.tile_pool(name="pb", bufs=1, space="PSUM"))

        # ---- constants / inputs, first-need-first DMA order ----
        ind16_sb = consts.tile([128, H], DT)
        indw_sb = consts.tile([64, M, 128], DT)
        nc.sync.dma_start(out=ind16_sb, in_=ind16.ap())
        nc.sync.dma_start(out=indw_sb, in_=indw.ap())

        # block-0 query input before the weights: it gates the first matmul
        q_in0 = p_inq.tile([128, NJ, TB], DT, tag="q", name="q")
        nc.sync.dma_start(out=q_in0, in_=qT_v[:, :, 0:TB])
        wq_sb = consts.tile([128, NJ, C], DT)
        for r in range(NJ):
            nc.sync.dma_start(
                out=wq_sb[:, :, r * 128:(r + 1) * 128],
                in_=wq_v[:, :, r * 128:(r + 1) * 128],
            )
        k_in0 = [p_ink.tile([128, NJ, TB], DT, tag="kin", name="kin")
                 for _ in range(M)]
        nc.sync.dma_start(out=k_in0[0], in_=kT_v[:, 0, :, 0:TB])
        wk_sb = consts.tile([128, NJ, C], DT)
        for r in range(NJ):
            nc.sync.dma_start(
                out=wk_sb[:, :, r * 128:(r + 1) * 128],
                in_=wk_v[:, :, r * 128:(r + 1) * 128],
            )
        for m in range(1, M):
            nc.sync.dma_start(out=k_in0[m], in_=kT_v[:, m, :, 0:TB])
        wo_sb = consts.tile([128, NJ, C], DT)
        nc.sync.dma_start(out=wo_sb, in_=wo_v)
        ident = consts.tile([128, 128], DT)
        make_identity(nc, ident)
        if with_bias:
            ones_sb = consts.tile([1, TB], DT)
            nc.vector.memset(ones_sb, 1.0)
            bq_sb = consts.tile([1, C], DT)
            bk_sb = consts.tile([1, C], DT)
            bo_sb = consts.tile([1, C], DT)
            nc.sync.dma_start(out=bq_sb, in_=bq.ap())
            nc.sync.dma_start(out=bk_sb, in_=bk.ap())
            nc.sync.dma_start(out=bo_sb, in_=bo.ap())

        for blk in range(nblk):
            t0 = blk * TB
            tsl = slice(t0, t0 + TB)

            # ---- load inputs (block 0 preloaded above) ----
            if blk == 0:
                q_in, k_in = q_in0, k_in0
            else:
                q_in = p_inq.tile([128, NJ, TB], DT, tag="q", name="q")
                nc.sync.dma_start(out=q_in, in_=qT_v[:, :, tsl])
                k_in = [p_ink.tile([128, NJ, TB], DT, tag="kin", name="kin")
                        for _ in range(M)]
                for m in range(M):
                    nc.sync.dma_start(out=k_in[m], in_=kT_v[:, m, :, tsl])

            # ---- Q projection (PE; psum evacuated by DVE) ----
            qp = [p_qp.tile([128, TB], DT, tag="qp", name="qp") for _ in range(NJ)]
            for r in range(NJ):
                ps = pp.tile([128, TB], F32, tag="pp", name="pp")
                for c in range(NJ):
                    nc.tensor.matmul(
                        ps,
                        wq_sb[:, c, r * 128:(r + 1) * 128],
                        q_in[:, c, :],
                        start=(c == 0),
                        stop=(c == NJ - 1 and not with_bias),
                    )
                if with_bias:
                    nc.tensor.matmul(
                        ps, bq_sb[:, r * 128:(r + 1) * 128], ones_sb,
                        start=False, stop=True,
                    )
                nc.vector.tensor_copy(qp[r], ps)

            # ---- K projection (PE), m-outer; per-(r,m-pair) kp tiles so
            #      the q*k product chains accumulate during the stream ----
            kp = [[None] * MP for _ in range(NJ)]
            Rs = []
            for m in range(M):
                mp, ml = m // 2, m % 2
                for r in range(NJ):
                    ps = pp.tile([128, TB], F32, tag="pp", name="pp")
                    for c in range(NJ):
                        nc.tensor.matmul(
                            ps,
                            wk_sb[:, c, r * 128:(r + 1) * 128],
                            k_in[m][:, c, :],
                            start=(c == 0),
                            stop=(c == NJ - 1 and not with_bias),
                        )
                    if with_bias:
                        nc.tensor.matmul(
                            ps, bk_sb[:, r * 128:(r + 1) * 128], ones_sb,
                            start=False, stop=True,
                        )
                    if ml == 0:
                        kp[r][mp] = p_kp.tile([128, 2, TB], DT, tag="kp",
                                              name="kp")
                    nc.scalar.copy(out=kp[r][mp][:, ml, :], in_=ps)

                # R_mp = sum_r qp_r * kp_r_mp  (DVE f16, fires as copies land)
                if ml == 1:
                    Rm = p_R.tile([128, 2, TB], DT, tag="R", name="R")
                    nc.vector.tensor_mul(Rm, _bcast(qp[0], 2, 1), kp[0][mp])
                    for r in range(1, NJ):
                        t = p_tmp.tile([128, 2, TB], DT, tag="tmp", name="tmp")
                        nc.vector.tensor_mul(t, _bcast(qp[r], 2, 1), kp[r][mp])
                        nc.vector.tensor_add(Rm, Rm, t)
                    Rs.append(Rm)

            # ---- attention logits, token-major: pslt[t, tt, m, h] ----
            pslt = pl.tile([128, NT, M, H], F32, tag="pl", name="pl")
            for m in range(M):
                for tt in range(NT):
                    nc.tensor.matmul(
                        pslt[:, tt, m, :],
                        Rs[m // 2][:, m % 2, tt * 128:(tt + 1) * 128],
                        ind16_sb,
                        start=True,
                        stop=True,
                    )

            # ---- softmax over M (token-major, full 128 partitions) ----
            e = p_sm.tile([128, NT, M, H], F32, tag="e", name="e")
            nc.scalar.activation(e, pslt, func=mybir.ActivationFunctionType.Exp)
            s01 = p_sm.tile([128, NT, H], F32, tag="s01", name="s01")
            s = p_sm.tile([128, NT, H], F32, tag="s", name="s")
            nc.vector.tensor_add(s01, e[:, :, 0, :], e[:, :, 1, :])
            nc.vector.tensor_add(s, e[:, :, 2, :], e[:, :, 3, :])
            nc.vector.tensor_add(s, s01, s)
            rcp = p_sm.tile([128, NT, H], F32, tag="rcp", name="rcp")
            nc.vector.reciprocal(rcp, s)
            w_t = p_sm.tile([128, NT, M, H], DT, tag="w", name="w")
            nc.vector.tensor_mul(w_t, e, _bcast(rcp, M, 2))

            # transpose w to head-major: wT[(m,h), (tt,t)]
            wT = p_wT.tile([64, NT, 128], DT, tag="wT", name="wT")
            for tt in range(NT):
                pst = pw.tile([64, 128], DT, tag="pw", name="pw")
                nc.tensor.transpose(pst, w_t[:, tt, :, :], ident)
                nc.scalar.copy(out=wT[:, tt, :], in_=pst)

            # ---- head->channel broadcast of softmax weights (PE, one MM per m) ----
            wbc = p_wbc.tile([128, M, TB], DT, tag="wbc", name="wbc")
            for m in range(M):
                psb = pb.tile([128, TB], F32, tag="pb", name="pb")
                nc.tensor.matmul(
                    psb, indw_sb[:, m, :], wT, start=True, stop=True,
                )
                nc.scalar.copy(out=wbc[:, m, :], in_=psb)

            # ---- weighted sum of k^2 (ACT square + DVE m-pair mul/adds) ----
            yb = [p_yb.tile([128, TB], DT, tag="yb", name="yb") for _ in range(NJ)]
            for r in range(NJ):
                u = [None, None]
                for mp in range(MP):
                    ksq = p_ksq.tile([128, 2, TB], DT, tag="ksq", name="ksq")
                    nc.scalar.activation(
                        ksq, kp[r][mp], func=mybir.ActivationFunctionType.Square
                    )
                    u[mp] = p_ct.tile([128, 2, TB], DT, tag="ct", name="ct")
                    nc.vector.tensor_mul(
                        u[mp], wbc[:, 2 * mp:2 * mp + 2, :], ksq
                    )
                nc.vector.tensor_add(u[0], u[0], u[1])
                nc.vector.tensor_add(yb[r], u[0][:, 0, :], u[0][:, 1, :])

            # ---- output projection (PE, own PSUM pool so the yb-gated
            #      accumulation groups never starve the projection stream) ----
            for tt in range(NT):
                for oc in range(2):
                    ps = po.tile([128, 512], F32, tag="po", name="po")
                    for r in range(NJ):
                        nc.tensor.matmul(
                            ps,
                            yb[r][:, tt * 128:(tt + 1) * 128],
                            wo_sb[:, r, oc * 512:(oc + 1) * 512],
                            start=(r == 0),
                            stop=(r == NJ - 1 and not with_bias),
                        )
                    if with_bias:
                        nc.tensor.matmul(
                            ps,
                            ones_sb[:, :128],
                            bo_sb[:, oc * 512:(oc + 1) * 512],
                            start=False,
                            stop=True,
                        )
                    o_sb = p_out.tile([128, 512], F32, tag="outs", name="osb")
                    nc.scalar.copy(out=o_sb, in_=ps)
                    nc.sync.dma_start(
                        out=out.ap()[t0 + tt * 128:t0 + (tt + 1) * 128,
                                     oc * 512:(oc + 1) * 512],
                        in_=o_sb,
                    )
    nc.compile()
    return nc


def _host_prep(query, key, Wq, Wk, Wo, bq, bk, bo):
    qT = np.ascontiguousarray(query.reshape(T_TOTAL, C).T).astype(NPDT)
    kT = np.ascontiguousarray(key.reshape(T_TOTAL, M, C).transpose(1, 2, 0)).astype(NPDT)

    # head-interleaved projection space: new channel c = d*16 + h
    cc = np.arange(C)
    old0 = (cc % H) * HD + cc // H
    wqT = np.ascontiguousarray(Wq.T[:, old0]).astype(NPDT)
    wkT = np.ascontiguousarray(Wk.T[:, old0]).astype(NPDT)
    woT = np.ascontiguousarray(Wo.T[old0, :]).astype(NPDT)

    p = np.arange(128)
    ind16 = (p[:, None] % H == np.arange(H)[None, :]).astype(NPDT) * NPDT(SCALE)
    # indw[q, m, p] = 1 iff q == m*H + (p % H)
    q_ = np.arange(64)[:, None, None]
    m_ = np.arange(M)[None, :, None]
    indw = (q_ == m_ * H + p[None, None, :] % H).astype(NPDT)

    with_bias = bool(np.any(bq) or np.any(bk) or np.any(bo))
    common = {"wqT": wqT, "wkT": wkT, "woT": woT, "ind16": ind16, "indw": indw}
    if with_bias:
        common |= {
            "bq": bq.reshape(1, C)[:, old0].astype(NPDT),
            "bk": bk.reshape(1, C)[:, old0].astype(NPDT),
            "bo": bo.reshape(1, C).astype(NPDT),
        }
    in_maps = []
    for i in range(NCORES):
        sl = slice(i * T_CORE, (i + 1) * T_CORE)
        in_maps.append(
            {
                "qT": np.ascontiguousarray(qT[:, sl]),
                "kT": np.ascontiguousarray(kT[:, :, sl]),
                **common,
            }
        )
    return in_maps, with_bias


_NC_CACHE = {}
_LAST_RESULT = None


def kernel(query, key, gate, Wq, bq, Wk, bk, Wv, bv, Wg, bg, Wo, bo):
    in_maps, with_bias = _host_prep(query, key, Wq, Wk, Wo, bq, bk, bo)
    key_ = (T_CORE, with_bias)
    if key_ not in _NC_CACHE:
        _NC_CACHE[key_] = build_nc(T_CORE, with_bias)
    nc = _NC_CACHE[key_]
    res = run_bass_kernel_spmd(nc, in_maps, list(range(NCORES)))
    global _LAST_RESULT
    _LAST_RESULT = res
    out = np.concatenate([res.results[i]["out"] for i in range(NCORES)], axis=0)
    return out.reshape(B, N, C)
